# revision 6
# baseline (speedup 1.0000x reference)
"""Trainium2 Bass kernel for a GPT-style transformer block.

Shapes (hardcoded): x [2, 2048, 1024], n_head=16, causal attention + GELU MLP.
Strategy: row-sharding (4096 rows -> 512 rows/core on 8 cores).
  Launch A: per-core LN1 + qkv projection for own rows -> qkvT [3072, 512] bf16.
  Host:     reassemble full K^T / V per batch, build per-core masks (no arithmetic).
  Launch B: per-core attention over own 512 query rows (full 2048-key range with
            data mask), proj, residual, LN2, FFN, residual -> out rows [512, 1024].
All matmuls bf16 with fp32 PSUM accumulation; residual stream / LN / softmax sums fp32.
"""

import sys

if "/opt/trn_rl_repo" not in sys.path:
    sys.path.insert(0, "/opt/trn_rl_repo")

import numpy as np
import ml_dtypes

import concourse.bacc as bacc
import concourse.tile as tile
from concourse import mybir
from concourse.bass_utils import run_bass_kernel_spmd

F32 = mybir.dt.float32
BF16 = mybir.dt.bfloat16
AF = mybir.ActivationFunctionType
BF = ml_dtypes.bfloat16

B, T, D = 2, 2048, 1024
H, DH = 16, 64
R = B * T          # 4096 flattened rows
NCORE = 8
RC = R // NCORE    # 512 rows per core
EPS = 1e-5
D3 = 3 * D         # 3072
DF = 4 * D         # 4096
VA = H * (DH + 1)  # 1040: V augmented with a ones column per head


def _layer_norm_tiles(nc, sb, x_tile, w_bc, b_bc, out_dtype):
    """LN over free axis of x_tile [128, D] fp32 -> new tile [128, D] out_dtype."""
    stats = sb.tile([128, 2, 6], F32, tag="ln_stats")
    xg = x_tile[:].rearrange("p (s d) -> p s d", s=2)
    for s in range(2):
        nc.vector.bn_stats(stats[:, s, :], xg[:, s, :])
    mv = sb.tile([128, 2], F32, tag="ln_mv")
    nc.vector.bn_aggr(mv[:], stats[:])
    rstd = sb.tile([128, 1], F32, tag="ln_rstd")
    eps_sb = sb.tile([128, 1], F32, tag="ln_eps")
    nc.vector.memset(eps_sb[:], EPS)
    nc.scalar.activation(rstd[:], mv[:, 1:2], AF.Sqrt, bias=eps_sb[:], scale=1.0)
    nc.vector.reciprocal(rstd[:], rstd[:])
    xn = sb.tile([128, D], F32, tag="ln_xn")
    nc.vector.tensor_scalar(
        xn[:], x_tile[:], mv[:, 0:1], rstd[:],
        mybir.AluOpType.subtract, mybir.AluOpType.mult,
    )
    nc.vector.tensor_mul(xn[:], xn[:], w_bc[:])
    out = sb.tile([128, D], out_dtype, tag="ln_out")
    nc.vector.tensor_add(out[:], xn[:], b_bc[:])
    return out


def _build_a():
    nc = bacc.Bacc("TRN2", target_bir_lowering=False, debug=False, num_devices=NCORE)
    x = nc.dram_tensor("x", [RC, D], F32, kind="ExternalInput")
    ln1w = nc.dram_tensor("ln1w", [D], F32, kind="ExternalInput")
    ln1b = nc.dram_tensor("ln1b", [D], F32, kind="ExternalInput")
    wattn = nc.dram_tensor("wattn", [D, D3], BF16, kind="ExternalInput")
    battn = nc.dram_tensor("battn", [D3], F32, kind="ExternalInput")
    qkvT = nc.dram_tensor("qkvT", [D3, RC], BF16, kind="ExternalOutput")

    with tile.TileContext(nc) as tc:
        with (
            tc.tile_pool(name="const", bufs=1) as const,
            tc.tile_pool(name="w", bufs=1) as wpool,
            tc.tile_pool(name="sb", bufs=3) as sb,
            tc.tile_pool(name="lt", bufs=1) as lt,
            tc.tile_pool(name="ps", bufs=4, space="PSUM") as ps,
            tc.tile_pool(name="outp", bufs=3) as outp,
        ):
            ln1w_bc = const.tile([128, D], F32, tag="ln1w")
            nc.sync.dma_start(ln1w_bc[:], ln1w.ap().partition_broadcast(128))
            ln1b_bc = const.tile([128, D], F32, tag="ln1b")
            nc.sync.dma_start(ln1b_bc[:], ln1b.ap().partition_broadcast(128))
            battn_all = const.tile([128, D3 // 128], F32, tag="battn")
            nc.sync.dma_start(battn_all[:], battn.ap().rearrange("(m p) -> p m", p=128))

            w_sb = [wpool.tile([128, D3], BF16, tag=f"w{k}", name=f"w{k}") for k in range(8)]
            for k in range(8):
                nc.sync.dma_start(w_sb[k][:], wattn[k * 128:(k + 1) * 128, :])

            ln1T = [lt.tile([128, RC], BF16, tag=f"ln1T{j}", name=f"ln1T{j}") for j in range(8)]
            for t in range(4):
                x_sb = sb.tile([128, D], F32, tag="x")
                nc.sync.dma_start(x_sb[:], x[t * 128:(t + 1) * 128, :])
                ln1n = _layer_norm_tiles(nc, sb, x_sb, ln1w_bc, ln1b_bc, BF16)
                for j in range(8):
                    nc.sync.dma_start_transpose(
                        ln1T[j][:, t * 128:(t + 1) * 128],
                        ln1n[:, j * 128:(j + 1) * 128],
                    )

            for m in range(D3 // 128):  # 24 output col-tiles
                psum = ps.tile([128, RC], F32, tag="mm")
                for k in range(8):
                    nc.tensor.matmul(
                        psum[:], w_sb[k][:, m * 128:(m + 1) * 128], ln1T[k][:],
                        start=(k == 0), stop=(k == 7),
                    )
                o_sb = outp.tile([128, RC], BF16, tag="o")
                nc.scalar.activation(
                    o_sb[:], psum[:], AF.Identity,
                    bias=battn_all[:, m:m + 1], scale=1.0,
                )
                nc.sync.dma_start(qkvT[m * 128:(m + 1) * 128, :], o_sb[:])

    nc.compile()
    return nc


def _build_b():
    nc = bacc.Bacc("TRN2", target_bir_lowering=False, debug=False, num_devices=NCORE)
    x = nc.dram_tensor("x", [RC, D], F32, kind="ExternalInput")
    qT = nc.dram_tensor("qT", [D, RC], BF16, kind="ExternalInput")
    kT = nc.dram_tensor("kT", [D, T], BF16, kind="ExternalInput")
    vaug = nc.dram_tensor("vaug", [T, VA], BF16, kind="ExternalInput")
    maskT = nc.dram_tensor("maskT", [T, RC], BF16, kind="ExternalInput")
    wproj = nc.dram_tensor("wproj", [D, D], BF16, kind="ExternalInput")
    bproj = nc.dram_tensor("bproj", [D], F32, kind="ExternalInput")
    ln2w = nc.dram_tensor("ln2w", [D], F32, kind="ExternalInput")
    ln2b = nc.dram_tensor("ln2b", [D], F32, kind="ExternalInput")
    wfc = nc.dram_tensor("wfc", [D, DF], BF16, kind="ExternalInput")
    bfc = nc.dram_tensor("bfc", [DF], F32, kind="ExternalInput")
    wout = nc.dram_tensor("wout", [DF, D], BF16, kind="ExternalInput")
    bout = nc.dram_tensor("bout", [D], F32, kind="ExternalInput")
    out = nc.dram_tensor("out", [RC, D], F32, kind="ExternalOutput")

    KT_TILES = T // 128  # 16 key tiles

    with tile.TileContext(nc) as tc:
        with (
            tc.tile_pool(name="const", bufs=1) as const,
            tc.tile_pool(name="att", bufs=1) as attp,
            tc.tile_pool(name="sb", bufs=2) as sb,
        ):
            bproj_bc = const.tile([128, D], F32, tag="bproj")
            nc.sync.dma_start(bproj_bc[:], bproj.ap().partition_broadcast(128))
            ln2w_bc = const.tile([128, D], F32, tag="ln2w")
            nc.sync.dma_start(ln2w_bc[:], ln2w.ap().partition_broadcast(128))
            ln2b_bc = const.tile([128, D], F32, tag="ln2b")
            nc.sync.dma_start(ln2b_bc[:], ln2b.ap().partition_broadcast(128))
            bout_bc = const.tile([128, D], F32, tag="bout")
            nc.sync.dma_start(bout_bc[:], bout.ap().partition_broadcast(128))
            bfc_all = const.tile([128, DF // 128], F32, tag="bfc")
            nc.sync.dma_start(bfc_all[:], bfc.ap().rearrange("(m p) -> p m", p=128))

            attT = [attp.tile([128, RC], BF16, tag=f"attT{i}", name=f"attT{i}") for i in range(8)]

            # ---------------- attention ----------------
            with (
                tc.tile_pool(name="kv", bufs=1) as kv,
                tc.tile_pool(name="exps", bufs=18) as exps,
                tc.tile_pool(name="aps", bufs=5, space="PSUM") as aps,
                tc.tile_pool(name="avps", bufs=2, space="PSUM") as avps,
                tc.tile_pool(name="asb", bufs=2) as asb,
            ):
                qT_sb = [kv.tile([128, RC], BF16, tag=f"qT{i}", name=f"qTs{i}") for i in range(8)]
                kT_sb = [kv.tile([128, T], BF16, tag=f"kT{i}", name=f"kTs{i}") for i in range(8)]
                v_sb = [kv.tile([128, VA], BF16, tag=f"v{i}", name=f"vs{i}") for i in range(KT_TILES)]
                m_sb = [kv.tile([128, RC], BF16, tag=f"m{i}", name=f"ms{i}") for i in range(KT_TILES)]
                for i in range(8):
                    nc.sync.dma_start(qT_sb[i][:], qT[i * 128:(i + 1) * 128, :])
                    nc.sync.dma_start(kT_sb[i][:], kT[i * 128:(i + 1) * 128, :])
                for i in range(KT_TILES):
                    nc.sync.dma_start(v_sb[i][:], vaug[i * 128:(i + 1) * 128, :])
                    nc.sync.dma_start(m_sb[i][:], maskT[i * 128:(i + 1) * 128, :])

                for h in range(H):
                    hp, hl = h // 2, (h % 2) * 64
                    e_tiles = []
                    for kt in range(KT_TILES):
                        s_ps = aps.tile([128, RC], F32, tag="s")
                        nc.tensor.matmul(
                            s_ps[:],
                            kT_sb[hp][hl:hl + 64, kt * 128:(kt + 1) * 128],
                            qT_sb[hp][hl:hl + 64, :],
                            start=True, stop=True,
                        )
                        e_sb = exps.tile([128, RC], BF16, tag="e")
                        nc.scalar.activation(e_sb[:], s_ps[:], AF.Exp,
                                             bias=0.0, scale=0.125)
                        nc.vector.tensor_mul(e_sb[:], e_sb[:], m_sb[kt][:])
                        e_tiles.append(e_sb)
                    av_ps = avps.tile([DH + 1, RC], F32, tag="av")
                    for kt in range(KT_TILES):
                        nc.tensor.matmul(
                            av_ps[:], v_sb[kt][:, h * 65:(h + 1) * 65], e_tiles[kt][:],
                            start=(kt == 0), stop=(kt == KT_TILES - 1),
                        )
                    r_sb = asb.tile([1, RC], F32, tag="r")
                    nc.vector.tensor_copy(r_sb[:], av_ps[DH:DH + 1, :])
                    nc.vector.reciprocal(r_sb[:], r_sb[:])
                    rb_sb = asb.tile([64, RC], F32, tag="rb")
                    nc.gpsimd.partition_broadcast(rb_sb[:], r_sb[:])
                    nc.vector.tensor_mul(
                        attT[hp][hl:hl + 64, :], av_ps[0:DH, :], rb_sb[:],
                    )

            # ---------------- proj + residual + LN2 ----------------
            x2p_cm = tc.tile_pool(name="x2p", bufs=1)
            x2p = x2p_cm.__enter__()
            x_sb = [x2p.tile([128, D], F32, tag=f"x{t}", name=f"x{t}") for t in range(4)]
            for t in range(4):
                nc.sync.dma_start(x_sb[t][:], x[t * 128:(t + 1) * 128, :])
            ln2T = [x2p.tile([128, RC], BF16, tag=f"ln2T{j}", name=f"ln2T{j}") for j in range(8)]
            x2_sb = [x2p.tile([128, D], F32, tag=f"x2{t}", name=f"x2{t}") for t in range(4)]
            with (
                tc.tile_pool(name="wpj", bufs=1) as wpj,
                tc.tile_pool(name="pps", bufs=4, space="PSUM") as pps,
            ):
                wp_sb = [wpj.tile([128, D], BF16, tag=f"wp{i}", name=f"wp{i}") for i in range(8)]
                for i in range(8):
                    nc.sync.dma_start(wp_sb[i][:], wproj[i * 128:(i + 1) * 128, :])
                for qm in range(4):
                    for oc in range(2):
                        y_ps = pps.tile([128, 512], F32, tag="y")
                        for cc in range(8):
                            nc.tensor.matmul(
                                y_ps[:],
                                attT[cc][:, qm * 128:(qm + 1) * 128],
                                wp_sb[cc][:, oc * 512:(oc + 1) * 512],
                                start=(cc == 0), stop=(cc == 7),
                            )
                        sl = slice(oc * 512, (oc + 1) * 512)
                        nc.vector.tensor_add(x2_sb[qm][:, sl], y_ps[:], x_sb[qm][:, sl])
                        nc.vector.tensor_add(
                            x2_sb[qm][:, sl], x2_sb[qm][:, sl], bproj_bc[:, sl],
                        )
                    ln2n = _layer_norm_tiles(nc, sb, x2_sb[qm], ln2w_bc, ln2b_bc, BF16)
                    for j in range(8):
                        nc.sync.dma_start_transpose(
                            ln2T[j][:, qm * 128:(qm + 1) * 128],
                            ln2n[:, j * 128:(j + 1) * 128],
                        )

            # ---------------- FFN ----------------
            with tc.tile_pool(name="g", bufs=1) as gp:
                g_sb = [gp.tile([128, RC], BF16, tag=f"g{i}", name=f"g{i}") for i in range(32)]
                with (
                    tc.tile_pool(name="wf", bufs=1) as wf,
                    tc.tile_pool(name="fps", bufs=4, space="PSUM") as fps,
                ):
                    wf_sb = [wf.tile([128, DF], BF16, tag=f"wf{i}", name=f"wfs{i}") for i in range(8)]
                    for i in range(8):
                        nc.sync.dma_start(wf_sb[i][:], wfc[i * 128:(i + 1) * 128, :])
                    for hm in range(32):
                        h_ps = fps.tile([128, RC], F32, tag="h")
                        for k in range(8):
                            nc.tensor.matmul(
                                h_ps[:], wf_sb[k][:, hm * 128:(hm + 1) * 128],
                                ln2T[k][:], start=(k == 0), stop=(k == 7),
                            )
                        nc.scalar.activation(
                            g_sb[hm][:], h_ps[:], AF.Gelu,
                            bias=bfc_all[:, hm:hm + 1], scale=1.0,
                        )

                with (
                    tc.tile_pool(name="wo", bufs=1) as wo,
                    tc.tile_pool(name="ops", bufs=4, space="PSUM") as ops,
                    tc.tile_pool(name="osb", bufs=3) as osb,
                ):
                    wo_sb = [wo.tile([128, D], BF16, tag=f"wo{i}", name=f"wos{i}") for i in range(32)]
                    for i in range(32):
                        nc.sync.dma_start(wo_sb[i][:], wout[i * 128:(i + 1) * 128, :])
                    for qm in range(4):
                        o_tile = osb.tile([128, D], F32, tag="o")
                        for oc in range(2):
                            o_ps = ops.tile([128, 512], F32, tag="ops")
                            for hh in range(32):
                                nc.tensor.matmul(
                                    o_ps[:],
                                    g_sb[hh][:, qm * 128:(qm + 1) * 128],
                                    wo_sb[hh][:, oc * 512:(oc + 1) * 512],
                                    start=(hh == 0), stop=(hh == 31),
                                )
                            sl = slice(oc * 512, (oc + 1) * 512)
                            nc.vector.tensor_add(o_tile[:, sl], o_ps[:], x2_sb[qm][:, sl])
                            nc.vector.tensor_add(o_tile[:, sl], o_tile[:, sl], bout_bc[:, sl])
                        nc.sync.dma_start(out[qm * 128:(qm + 1) * 128, :], o_tile[:])
            x2p_cm.__exit__(None, None, None)

    nc.compile()
    return nc


_CACHE = {}


def _get(name, builder):
    if name not in _CACHE:
        _CACHE[name] = builder()
    return _CACHE[name]


def kernel(x, ln1_w, ln1_b, ln2_w, ln2_b, w_attn, b_attn, w_proj, b_proj,
           w_fc, b_fc, w_out, b_out):
    x = np.asarray(x, np.float32)
    xf = np.ascontiguousarray(x.reshape(R, D))
    wattn_bf = np.asarray(w_attn, np.float32).astype(BF)
    cores = list(range(NCORE))

    nc_a = _get("a", _build_a)
    in_maps_a = [
        dict(
            x=xf[c * RC:(c + 1) * RC],
            ln1w=np.asarray(ln1_w, np.float32), ln1b=np.asarray(ln1_b, np.float32),
            wattn=wattn_bf, battn=np.asarray(b_attn, np.float32),
        )
        for c in cores
    ]
    res_a = run_bass_kernel_spmd(nc_a, in_maps_a, cores).results
    qkvT = [np.asarray(res_a[c]["qkvT"]) for c in cores]  # [3072, 512] bf16

    # host reassembly (pure data movement / layout)
    kT_b = [np.concatenate([qkvT[4 * b + r][D:2 * D] for r in range(4)], axis=1)
            for b in range(B)]                            # [1024, 2048] bf16
    vT_b = [np.concatenate([qkvT[4 * b + r][2 * D:3 * D] for r in range(4)], axis=1)
            for b in range(B)]
    vaug_b = []
    for b in range(B):
        vn = np.ascontiguousarray(vT_b[b].T)              # [2048, 1024]
        va = np.empty((T, H, DH + 1), dtype=BF)
        va[:, :, :DH] = vn.reshape(T, H, DH)
        va[:, :, DH] = np.ones((), dtype=BF)
        vaug_b.append(va.reshape(T, VA))

    key_idx = np.arange(T)
    nc_b = _get("b", _build_b)
    in_maps_b = []
    for c in cores:
        b = c // 4
        qb = (c % 4) * 512 + np.arange(RC)                # batch-local query rows
        maskT = (key_idx[:, None] <= qb[None, :]).astype(BF)
        in_maps_b.append(dict(
            x=xf[c * RC:(c + 1) * RC],
            qT=np.ascontiguousarray(qkvT[c][0:D]),
            kT=np.ascontiguousarray(kT_b[b]),
            vaug=vaug_b[b],
            maskT=maskT,
            wproj=np.asarray(w_proj, np.float32).astype(BF),
            bproj=np.asarray(b_proj, np.float32),
            ln2w=np.asarray(ln2_w, np.float32), ln2b=np.asarray(ln2_b, np.float32),
            wfc=np.asarray(w_fc, np.float32).astype(BF),
            bfc=np.asarray(b_fc, np.float32),
            wout=np.asarray(w_out, np.float32).astype(BF),
            bout=np.asarray(b_out, np.float32),
        ))
    res_b = run_bass_kernel_spmd(nc_b, in_maps_b, cores).results
    out = np.concatenate([np.asarray(res_b[c]["out"], np.float32) for c in cores], axis=0)
    return out.reshape(B, T, D)


# revision 9
# speedup vs baseline: 1.2152x; 1.2152x over previous
"""Trainium2 Bass kernel for a GPT-style transformer block.

Shapes (hardcoded): x [2, 2048, 1024], n_head=16, causal attention + GELU MLP.
Strategy: row-sharding (4096 rows -> 512 rows/core on 8 cores).
  Launch A: per-core LN1 + qkv projection for own rows -> qkvT [3072, 512] bf16.
  Host:     reassemble full K^T / V per batch, build per-core masks (no arithmetic).
  Launch B: per-core attention over own 512 query rows (full 2048-key range with
            data mask), proj, residual, LN2, FFN, residual -> out rows [512, 1024].
All matmuls bf16 with fp32 PSUM accumulation; residual stream / LN / softmax sums fp32.
Attention computes scores for head pairs row-packed on the PE (K=64 halves at
tile_position (0,0)/(64,0)) into one [128, 1024] PSUM span so exp runs on big tiles.
"""

import sys

if "/opt/trn_rl_repo" not in sys.path:
    sys.path.insert(0, "/opt/trn_rl_repo")

import numpy as np
import ml_dtypes

import concourse.bacc as bacc
import concourse.tile as tile
from concourse import mybir
from concourse.bass_utils import run_bass_kernel_spmd

F32 = mybir.dt.float32
BF16 = mybir.dt.bfloat16
AF = mybir.ActivationFunctionType
ALU = mybir.AluOpType
BF = ml_dtypes.bfloat16

B, T, D = 2, 2048, 1024
H, DH = 16, 64
R = B * T          # 4096 flattened rows
NCORE = 8
RC = R // NCORE    # 512 rows per core
EPS = 1e-5
D3 = 3 * D         # 3072
DF = 4 * D         # 4096
VA = H * (DH + 1)  # 1040: V augmented with a ones column per head


def _layer_norm_tiles(nc, sb, x_tile, w_bc, b_bc, out_dtype):
    """LN over free axis of x_tile [128, D] fp32 -> new tile [128, D] out_dtype."""
    stats = sb.tile([128, 2, 6], F32, tag="ln_stats")
    xg = x_tile[:].rearrange("p (s d) -> p s d", s=2)
    for s in range(2):
        nc.vector.bn_stats(stats[:, s, :], xg[:, s, :])
    mv = sb.tile([128, 2], F32, tag="ln_mv")
    nc.vector.bn_aggr(mv[:], stats[:])
    rstd = sb.tile([128, 1], F32, tag="ln_rstd")
    eps_sb = sb.tile([128, 1], F32, tag="ln_eps")
    nc.vector.memset(eps_sb[:], EPS)
    nc.scalar.activation(rstd[:], mv[:, 1:2], AF.Sqrt, bias=eps_sb[:], scale=1.0)
    nc.vector.reciprocal(rstd[:], rstd[:])
    xn = sb.tile([128, D], F32, tag="ln_xn")
    nc.vector.tensor_scalar(
        xn[:], x_tile[:], mv[:, 0:1], rstd[:], ALU.subtract, ALU.mult,
    )
    nc.vector.tensor_mul(xn[:], xn[:], w_bc[:])
    out = sb.tile([128, D], out_dtype, tag="ln_out")
    nc.vector.tensor_add(out[:], xn[:], b_bc[:])
    return out


def _spread_transpose(nc, dst_tiles, src_tile, t, idx0=0):
    """Transpose src [128, D] bf16 into dst_tiles[j][:, t*128:(t+1)*128],
    alternating between the two HWDGE queues (sync / scalar)."""
    for j in range(8):
        eng = nc.sync if (idx0 + j) % 2 == 0 else nc.scalar
        eng.dma_start_transpose(
            dst_tiles[j][:, t * 128:(t + 1) * 128],
            src_tile[:, j * 128:(j + 1) * 128],
        )


def _build_a():
    nc = bacc.Bacc("TRN2", target_bir_lowering=False, debug=False, num_devices=NCORE)
    x = nc.dram_tensor("x", [RC, D], F32, kind="ExternalInput")
    ln1w = nc.dram_tensor("ln1w", [D], F32, kind="ExternalInput")
    ln1b = nc.dram_tensor("ln1b", [D], F32, kind="ExternalInput")
    wattn = nc.dram_tensor("wattn", [D, D3], BF16, kind="ExternalInput")
    battn = nc.dram_tensor("battn", [D3], F32, kind="ExternalInput")
    qkvT = nc.dram_tensor("qkvT", [D3, RC], BF16, kind="ExternalOutput")

    with tile.TileContext(nc) as tc:
        with (
            tc.tile_pool(name="const", bufs=1) as const,
            tc.tile_pool(name="w", bufs=1) as wpool,
            tc.tile_pool(name="sb", bufs=2) as sb,
            tc.tile_pool(name="lt", bufs=1) as lt,
            tc.tile_pool(name="ps", bufs=4, space="PSUM") as ps,
            tc.tile_pool(name="outp", bufs=3) as outp,
        ):
            ln1w_bc = const.tile([128, D], F32, tag="ln1w")
            nc.sync.dma_start(ln1w_bc[:], ln1w.ap().partition_broadcast(128))
            ln1b_bc = const.tile([128, D], F32, tag="ln1b")
            nc.sync.dma_start(ln1b_bc[:], ln1b.ap().partition_broadcast(128))
            battn_all = const.tile([128, D3 // 128], F32, tag="battn")
            nc.sync.dma_start(battn_all[:], battn.ap().rearrange("(m p) -> p m", p=128))

            w_sb = [wpool.tile([128, D3], BF16, tag=f"w{k}", name=f"w{k}") for k in range(8)]
            for k in range(8):
                nc.sync.dma_start(w_sb[k][:], wattn[k * 128:(k + 1) * 128, :])

            ln1T = [lt.tile([128, RC], BF16, tag=f"ln1T{j}", name=f"ln1T{j}") for j in range(8)]
            for t in range(4):
                x_sb = sb.tile([128, D], F32, tag="x")
                nc.sync.dma_start(x_sb[:], x[t * 128:(t + 1) * 128, :])
                ln1n = _layer_norm_tiles(nc, sb, x_sb, ln1w_bc, ln1b_bc, BF16)
                _spread_transpose(nc, ln1T, ln1n, t, idx0=t)

            for m in range(D3 // 128):  # 24 output col-tiles
                psum = ps.tile([128, RC], F32, tag="mm")
                for k in range(8):
                    nc.tensor.matmul(
                        psum[:], w_sb[k][:, m * 128:(m + 1) * 128], ln1T[k][:],
                        start=(k == 0), stop=(k == 7),
                    )
                o_sb = outp.tile([128, RC], BF16, tag="o")
                nc.scalar.activation(
                    o_sb[:], psum[:], AF.Identity,
                    bias=battn_all[:, m:m + 1], scale=1.0,
                )
                nc.sync.dma_start(qkvT[m * 128:(m + 1) * 128, :], o_sb[:])

    nc.compile()
    return nc


def _build_b():
    nc = bacc.Bacc("TRN2", target_bir_lowering=False, debug=False, num_devices=NCORE)
    x = nc.dram_tensor("x", [RC, D], F32, kind="ExternalInput")
    qT = nc.dram_tensor("qT", [D, RC], BF16, kind="ExternalInput")
    kT = nc.dram_tensor("kT", [D, T], BF16, kind="ExternalInput")
    vaug = nc.dram_tensor("vaug", [T, VA], BF16, kind="ExternalInput")
    maskT = nc.dram_tensor("maskT", [T, RC], BF16, kind="ExternalInput")
    wproj = nc.dram_tensor("wproj", [D, D], BF16, kind="ExternalInput")
    bproj = nc.dram_tensor("bproj", [D], F32, kind="ExternalInput")
    ln2w = nc.dram_tensor("ln2w", [D], F32, kind="ExternalInput")
    ln2b = nc.dram_tensor("ln2b", [D], F32, kind="ExternalInput")
    wfc = nc.dram_tensor("wfc", [D, DF], BF16, kind="ExternalInput")
    bfc = nc.dram_tensor("bfc", [DF], F32, kind="ExternalInput")
    wout = nc.dram_tensor("wout", [DF, D], BF16, kind="ExternalInput")
    bout = nc.dram_tensor("bout", [D], F32, kind="ExternalInput")
    out = nc.dram_tensor("out", [RC, D], F32, kind="ExternalOutput")

    KT_TILES = T // 128  # 16 key tiles

    with tile.TileContext(nc) as tc:
        with (
            tc.tile_pool(name="const", bufs=1) as const,
            tc.tile_pool(name="att", bufs=1) as attp,
            tc.tile_pool(name="sb", bufs=2) as sb,
        ):
            bproj_bc = const.tile([128, D], F32, tag="bproj")
            nc.sync.dma_start(bproj_bc[:], bproj.ap().partition_broadcast(128))
            ln2w_bc = const.tile([128, D], F32, tag="ln2w")
            nc.sync.dma_start(ln2w_bc[:], ln2w.ap().partition_broadcast(128))
            ln2b_bc = const.tile([128, D], F32, tag="ln2b")
            nc.sync.dma_start(ln2b_bc[:], ln2b.ap().partition_broadcast(128))
            bout_bc = const.tile([128, D], F32, tag="bout")
            nc.sync.dma_start(bout_bc[:], bout.ap().partition_broadcast(128))
            bfc_all = const.tile([128, DF // 128], F32, tag="bfc")
            nc.sync.dma_start(bfc_all[:], bfc.ap().rearrange("(m p) -> p m", p=128))

            attT = [attp.tile([128, RC], BF16, tag=f"attT{i}", name=f"attT{i}") for i in range(8)]

            # ---------------- attention ----------------
            with (
                tc.tile_pool(name="kv", bufs=1) as kv,
                tc.tile_pool(name="exps", bufs=18) as exps,
                tc.tile_pool(name="aps", bufs=3, space="PSUM") as aps,
                tc.tile_pool(name="avps", bufs=2, space="PSUM") as avps,
                tc.tile_pool(name="asb", bufs=3) as asb,
            ):
                qT_sb = [kv.tile([128, RC], BF16, tag=f"qT{i}", name=f"qTs{i}") for i in range(8)]
                kT_sb = [kv.tile([128, T], BF16, tag=f"kT{i}", name=f"kTs{i}") for i in range(8)]
                v_sb = [kv.tile([128, VA], BF16, tag=f"v{i}", name=f"vs{i}") for i in range(KT_TILES)]
                m_sb = [kv.tile([128, RC], BF16, tag=f"m{i}", name=f"ms{i}") for i in range(KT_TILES)]
                for i in range(8):
                    nc.sync.dma_start(qT_sb[i][:], qT[i * 128:(i + 1) * 128, :])
                    nc.scalar.dma_start(kT_sb[i][:], kT[i * 128:(i + 1) * 128, :])
                for i in range(KT_TILES):
                    nc.sync.dma_start(v_sb[i][:], vaug[i * 128:(i + 1) * 128, :])
                    nc.scalar.dma_start(m_sb[i][:], maskT[i * 128:(i + 1) * 128, :])

                for hp in range(H // 2):  # head pairs (2hp, 2hp+1)
                    e_tiles = []
                    for kt in range(KT_TILES):
                        s_ps = aps.tile([128, 2 * RC], F32, tag="s")
                        nc.tensor.matmul(
                            s_ps[:, 0:RC],
                            kT_sb[hp][0:64, kt * 128:(kt + 1) * 128],
                            qT_sb[hp][0:64, :],
                            start=True, stop=True, tile_position=(0, 0),
                        )
                        nc.tensor.matmul(
                            s_ps[:, RC:2 * RC],
                            kT_sb[hp][64:128, kt * 128:(kt + 1) * 128],
                            qT_sb[hp][64:128, :],
                            start=True, stop=True, tile_position=(64, 0),
                        )
                        e_sb = exps.tile([128, 2 * RC], BF16, tag="e")
                        nc.scalar.activation(e_sb[:], s_ps[:], AF.Exp,
                                             bias=0.0, scale=0.125)
                        # causal mask (multiplicative)
                        nc.vector.tensor_mul(e_sb[:, 0:RC], e_sb[:, 0:RC], m_sb[kt][:])
                        nc.vector.tensor_mul(e_sb[:, RC:2 * RC], e_sb[:, RC:2 * RC], m_sb[kt][:])
                        e_tiles.append(e_sb)
                    for half in range(2):
                        h = 2 * hp + half
                        hl = half * 64
                        av_ps = avps.tile([DH + 1, RC], F32, tag="av")
                        for kt in range(KT_TILES):
                            nc.tensor.matmul(
                                av_ps[:], v_sb[kt][:, h * 65:(h + 1) * 65],
                                e_tiles[kt][:, half * RC:(half + 1) * RC],
                                start=(kt == 0), stop=(kt == KT_TILES - 1),
                            )
                        r_sb = asb.tile([1, RC], F32, tag="r")
                        nc.vector.tensor_copy(r_sb[:], av_ps[DH:DH + 1, :])
                        rb_sb = asb.tile([64, RC], F32, tag="rb")
                        nc.gpsimd.partition_broadcast(rb_sb[:], r_sb[:])
                        nc.vector.reciprocal(rb_sb[:], rb_sb[:])
                        nc.vector.tensor_mul(
                            attT[hp][hl:hl + 64, :], av_ps[0:DH, :], rb_sb[:],
                        )

            # ---------------- proj + residual + LN2 ----------------
            x2p_cm = tc.tile_pool(name="x2p", bufs=1)
            x2p = x2p_cm.__enter__()
            x_sb = [x2p.tile([128, D], F32, tag=f"x{t}", name=f"x{t}") for t in range(4)]
            for t in range(4):
                nc.sync.dma_start(x_sb[t][:], x[t * 128:(t + 1) * 128, :])
            ln2T = [x2p.tile([128, RC], BF16, tag=f"ln2T{j}", name=f"ln2T{j}") for j in range(8)]
            x2_sb = [x2p.tile([128, D], F32, tag=f"x2{t}", name=f"x2{t}") for t in range(4)]
            with (
                tc.tile_pool(name="wpj", bufs=1) as wpj,
                tc.tile_pool(name="pps", bufs=4, space="PSUM") as pps,
            ):
                wp_sb = [wpj.tile([128, D], BF16, tag=f"wp{i}", name=f"wp{i}") for i in range(8)]
                for i in range(8):
                    nc.sync.dma_start(wp_sb[i][:], wproj[i * 128:(i + 1) * 128, :])
                for qm in range(4):
                    for oc in range(2):
                        y_ps = pps.tile([128, 512], F32, tag="y")
                        for cc in range(8):
                            nc.tensor.matmul(
                                y_ps[:],
                                attT[cc][:, qm * 128:(qm + 1) * 128],
                                wp_sb[cc][:, oc * 512:(oc + 1) * 512],
                                start=(cc == 0), stop=(cc == 7),
                            )
                        sl = slice(oc * 512, (oc + 1) * 512)
                        nc.vector.tensor_add(x2_sb[qm][:, sl], y_ps[:], x_sb[qm][:, sl])
                        nc.vector.tensor_add(
                            x2_sb[qm][:, sl], x2_sb[qm][:, sl], bproj_bc[:, sl],
                        )
                    ln2n = _layer_norm_tiles(nc, sb, x2_sb[qm], ln2w_bc, ln2b_bc, BF16)
                    _spread_transpose(nc, ln2T, ln2n, qm, idx0=qm)

            # ---------------- FFN ----------------
            with tc.tile_pool(name="g", bufs=1) as gp:
                g_sb = [gp.tile([128, RC], BF16, tag=f"g{i}", name=f"g{i}") for i in range(32)]
                with (
                    tc.tile_pool(name="wf", bufs=1) as wf,
                    tc.tile_pool(name="fps", bufs=4, space="PSUM") as fps,
                ):
                    wf_sb = [wf.tile([128, DF], BF16, tag=f"wf{i}", name=f"wfs{i}") for i in range(8)]
                    for i in range(8):
                        nc.sync.dma_start(wf_sb[i][:], wfc[i * 128:(i + 1) * 128, :])
                    for hm in range(32):
                        h_ps = fps.tile([128, RC], F32, tag="h")
                        for k in range(8):
                            nc.tensor.matmul(
                                h_ps[:], wf_sb[k][:, hm * 128:(hm + 1) * 128],
                                ln2T[k][:], start=(k == 0), stop=(k == 7),
                            )
                        nc.scalar.activation(
                            g_sb[hm][:], h_ps[:], AF.Gelu,
                            bias=bfc_all[:, hm:hm + 1], scale=1.0,
                        )

                with (
                    tc.tile_pool(name="wo", bufs=1) as wo,
                    tc.tile_pool(name="ops", bufs=4, space="PSUM") as ops,
                    tc.tile_pool(name="osb", bufs=3) as osb,
                ):
                    wo_sb = [wo.tile([128, D], BF16, tag=f"wo{i}", name=f"wos{i}") for i in range(32)]
                    for i in range(32):
                        nc.sync.dma_start(wo_sb[i][:], wout[i * 128:(i + 1) * 128, :])
                    for qm in range(4):
                        o_tile = osb.tile([128, D], F32, tag="o")
                        for oc in range(2):
                            o_ps = ops.tile([128, 512], F32, tag="ops")
                            for hh in range(32):
                                nc.tensor.matmul(
                                    o_ps[:],
                                    g_sb[hh][:, qm * 128:(qm + 1) * 128],
                                    wo_sb[hh][:, oc * 512:(oc + 1) * 512],
                                    start=(hh == 0), stop=(hh == 31),
                                )
                            sl = slice(oc * 512, (oc + 1) * 512)
                            nc.vector.tensor_add(o_tile[:, sl], o_ps[:], x2_sb[qm][:, sl])
                            nc.vector.tensor_add(o_tile[:, sl], o_tile[:, sl], bout_bc[:, sl])
                        nc.sync.dma_start(out[qm * 128:(qm + 1) * 128, :], o_tile[:])
            x2p_cm.__exit__(None, None, None)

    nc.compile()
    return nc


_CACHE = {}


def _get(name, builder):
    if name not in _CACHE:
        _CACHE[name] = builder()
    return _CACHE[name]


def kernel(x, ln1_w, ln1_b, ln2_w, ln2_b, w_attn, b_attn, w_proj, b_proj,
           w_fc, b_fc, w_out, b_out):
    x = np.asarray(x, np.float32)
    xf = np.ascontiguousarray(x.reshape(R, D))
    wattn_bf = np.asarray(w_attn, np.float32).astype(BF)
    cores = list(range(NCORE))

    nc_a = _get("a", _build_a)
    in_maps_a = [
        dict(
            x=xf[c * RC:(c + 1) * RC],
            ln1w=np.asarray(ln1_w, np.float32), ln1b=np.asarray(ln1_b, np.float32),
            wattn=wattn_bf, battn=np.asarray(b_attn, np.float32),
        )
        for c in cores
    ]
    res_a = run_bass_kernel_spmd(nc_a, in_maps_a, cores).results
    qkvT = [np.asarray(res_a[c]["qkvT"]) for c in cores]  # [3072, 512] bf16

    # host reassembly (pure data movement / layout)
    kT_b = [np.concatenate([qkvT[4 * b + r][D:2 * D] for r in range(4)], axis=1)
            for b in range(B)]                            # [1024, 2048] bf16
    vT_b = [np.concatenate([qkvT[4 * b + r][2 * D:3 * D] for r in range(4)], axis=1)
            for b in range(B)]
    vaug_b = []
    for b in range(B):
        vn = np.ascontiguousarray(vT_b[b].T)              # [2048, 1024]
        va = np.empty((T, H, DH + 1), dtype=BF)
        va[:, :, :DH] = vn.reshape(T, H, DH)
        va[:, :, DH] = np.ones((), dtype=BF)
        vaug_b.append(va.reshape(T, VA))

    key_idx = np.arange(T)
    nc_b = _get("b", _build_b)
    in_maps_b = []
    for c in cores:
        b = c // 4
        qb = (c % 4) * 512 + np.arange(RC)                # batch-local query rows
        maskT = (key_idx[:, None] <= qb[None, :]).astype(BF)
        in_maps_b.append(dict(
            x=xf[c * RC:(c + 1) * RC],
            qT=np.ascontiguousarray(qkvT[c][0:D]),
            kT=np.ascontiguousarray(kT_b[b]),
            vaug=vaug_b[b],
            maskT=maskT,
            wproj=np.asarray(w_proj, np.float32).astype(BF),
            bproj=np.asarray(b_proj, np.float32),
            ln2w=np.asarray(ln2_w, np.float32), ln2b=np.asarray(ln2_b, np.float32),
            wfc=np.asarray(w_fc, np.float32).astype(BF),
            bfc=np.asarray(b_fc, np.float32),
            wout=np.asarray(w_out, np.float32).astype(BF),
            bout=np.asarray(b_out, np.float32),
        ))
    res_b = run_bass_kernel_spmd(nc_b, in_maps_b, cores).results
    out = np.concatenate([np.asarray(res_b[c]["out"], np.float32) for c in cores], axis=0)
    return out.reshape(B, T, D)


# revision 17
# speedup vs baseline: 1.2814x; 1.0546x over previous
"""Trainium2 Bass kernel for a GPT-style transformer block.

Shapes (hardcoded): x [2, 2048, 1024], n_head=16, causal attention + GELU MLP.
Strategy: row-sharding (4096 rows -> 512 rows/core on 8 cores).
  Launch A: per-core LN1 + qkv projection for own rows -> qkvT [3072, 512] bf16.
  Host:     reassemble full K^T / V per batch, build per-core masks (no arithmetic).
  Launch B: per-core attention over own 512 query rows (full 2048-key range with
            data mask), proj, residual, LN2, FFN, residual -> out rows [512, 1024].
All matmuls bf16 with fp32 PSUM accumulation; residual stream / LN / softmax sums fp32.
Attention computes scores for head pairs row-packed on the PE (K=64 halves at
tile_position (0,0)/(64,0)) into one [128, 1024] PSUM span so exp runs on big tiles.
"""

import sys

if "/opt/trn_rl_repo" not in sys.path:
    sys.path.insert(0, "/opt/trn_rl_repo")

import numpy as np
import ml_dtypes

import concourse.bacc as bacc
import concourse.tile as tile
from concourse import masks
from concourse import mybir
from concourse.bass_utils import run_bass_kernel_spmd

F32 = mybir.dt.float32
BF16 = mybir.dt.bfloat16
AF = mybir.ActivationFunctionType
ALU = mybir.AluOpType
BF = ml_dtypes.bfloat16

B, T, D = 2, 2048, 1024
H, DH = 16, 64
R = B * T          # 4096 flattened rows
NCORE = 8
RC = R // NCORE    # 512 rows per core
EPS = 1e-5
D3 = 3 * D         # 3072
DF = 4 * D         # 4096
VA = H * (DH + 1)  # 1040: V augmented with a ones column per head


def _layer_norm_tiles(nc, sb, x_tile, w_bc, b_bc, out_dtype):
    """LN over free axis of x_tile [128, D] fp32 -> new tile [128, D] out_dtype."""
    stats = sb.tile([128, 2, 6], F32, tag="ln_stats")
    xg = x_tile[:].rearrange("p (s d) -> p s d", s=2)
    for s in range(2):
        nc.vector.bn_stats(stats[:, s, :], xg[:, s, :])
    mv = sb.tile([128, 2], F32, tag="ln_mv")
    nc.vector.bn_aggr(mv[:], stats[:])
    rstd = sb.tile([128, 1], F32, tag="ln_rstd")
    eps_sb = sb.tile([128, 1], F32, tag="ln_eps")
    nc.vector.memset(eps_sb[:], EPS)
    nc.scalar.activation(rstd[:], mv[:, 1:2], AF.Sqrt, bias=eps_sb[:], scale=1.0)
    nc.vector.reciprocal(rstd[:], rstd[:])
    xn = sb.tile([128, D], F32, tag="ln_xn")
    nc.vector.tensor_scalar(
        xn[:], x_tile[:], mv[:, 0:1], rstd[:], ALU.subtract, ALU.mult,
    )
    nc.vector.tensor_mul(xn[:], xn[:], w_bc[:])
    out = sb.tile([128, D], out_dtype, tag="ln_out")
    nc.vector.tensor_add(out[:], xn[:], b_bc[:])
    return out


def _pe_transpose(nc, tps, dst_tiles, src_tile, t, ident):
    """Transpose src [128, D] bf16 into dst_tiles[j][:, t*128:(t+1)*128]
    via the PE transpose path (idle during prep) + DVE copy out of PSUM."""
    for j in range(8):
        tp = tps.tile([128, 128], BF16, tag="tp")
        nc.tensor.transpose(tp[:], src_tile[:, j * 128:(j + 1) * 128], ident[:])
        nc.vector.tensor_copy(dst_tiles[j][:, t * 128:(t + 1) * 128], tp[:])


def _build_a():
    nc = bacc.Bacc("TRN2", target_bir_lowering=False, debug=False, num_devices=NCORE)
    x = nc.dram_tensor("x", [RC, D], F32, kind="ExternalInput")
    ln1w = nc.dram_tensor("ln1w", [D], F32, kind="ExternalInput")
    ln1b = nc.dram_tensor("ln1b", [D], F32, kind="ExternalInput")
    wattn = nc.dram_tensor("wattn", [D, D3], BF16, kind="ExternalInput")
    battn = nc.dram_tensor("battn", [D3], F32, kind="ExternalInput")
    qkvT = nc.dram_tensor("qkvT", [D3, RC], BF16, kind="ExternalOutput")

    with tile.TileContext(nc) as tc:
        with (
            tc.tile_pool(name="const", bufs=1) as const,
            tc.tile_pool(name="w", bufs=1) as wpool,
            tc.tile_pool(name="sb", bufs=2) as sb,
            tc.tile_pool(name="lt", bufs=1) as lt,
            tc.tile_pool(name="ps", bufs=4, space="PSUM") as ps,
            tc.tile_pool(name="tps", bufs=3, space="PSUM") as tps,
            tc.tile_pool(name="outp", bufs=3) as outp,
        ):
            ident = const.tile([128, 128], BF16, tag="ident")
            masks.make_identity(nc, ident[:])
            ln1w_bc = const.tile([128, D], F32, tag="ln1w")
            nc.sync.dma_start(ln1w_bc[:], ln1w.ap().partition_broadcast(128))
            ln1b_bc = const.tile([128, D], F32, tag="ln1b")
            nc.sync.dma_start(ln1b_bc[:], ln1b.ap().partition_broadcast(128))
            battn_all = const.tile([128, D3 // 128], F32, tag="battn")
            nc.sync.dma_start(battn_all[:], battn.ap().rearrange("(m p) -> p m", p=128))

            w_sb = [wpool.tile([128, D3], BF16, tag=f"w{k}", name=f"w{k}") for k in range(8)]
            for k in range(8):
                eng = nc.sync if k % 2 == 0 else nc.scalar
                eng.dma_start(w_sb[k][:], wattn[k * 128:(k + 1) * 128, :])

            ln1T = [lt.tile([128, RC], BF16, tag=f"ln1T{j}", name=f"ln1T{j}") for j in range(8)]
            for t in range(4):
                x_sb = sb.tile([128, D], F32, tag="x")
                nc.scalar.dma_start(x_sb[:], x[t * 128:(t + 1) * 128, :])
                ln1n = _layer_norm_tiles(nc, sb, x_sb, ln1w_bc, ln1b_bc, BF16)
                _pe_transpose(nc, tps, ln1T, ln1n, t, ident)

            for m in range(D3 // 128):  # 24 output col-tiles
                psum = ps.tile([128, RC], F32, tag="mm")
                for k in range(8):
                    nc.tensor.matmul(
                        psum[:], w_sb[k][:, m * 128:(m + 1) * 128], ln1T[k][:],
                        start=(k == 0), stop=(k == 7),
                    )
                o_sb = outp.tile([128, RC], BF16, tag="o")
                nc.scalar.activation(
                    o_sb[:], psum[:], AF.Identity,
                    bias=battn_all[:, m:m + 1], scale=1.0,
                )
                nc.sync.dma_start(qkvT[m * 128:(m + 1) * 128, :], o_sb[:])

    nc.compile()
    return nc


def _build_b():
    nc = bacc.Bacc("TRN2", target_bir_lowering=False, debug=False, num_devices=NCORE)
    x = nc.dram_tensor("x", [RC, D], F32, kind="ExternalInput")
    qT = nc.dram_tensor("qT", [D, RC], BF16, kind="ExternalInput")
    kT = nc.dram_tensor("kT", [D, T], BF16, kind="ExternalInput")
    vaug = nc.dram_tensor("vaug", [T, VA], BF16, kind="ExternalInput")
    maskT = nc.dram_tensor("maskT", [T, RC], BF16, kind="ExternalInput")
    wproj = nc.dram_tensor("wproj", [D, D], BF16, kind="ExternalInput")
    bproj = nc.dram_tensor("bproj", [D], F32, kind="ExternalInput")
    ln2w = nc.dram_tensor("ln2w", [D], F32, kind="ExternalInput")
    ln2b = nc.dram_tensor("ln2b", [D], F32, kind="ExternalInput")
    wfc = nc.dram_tensor("wfc", [D, DF], BF16, kind="ExternalInput")
    bfc = nc.dram_tensor("bfc", [DF], F32, kind="ExternalInput")
    wout = nc.dram_tensor("wout", [DF, D], BF16, kind="ExternalInput")
    bout = nc.dram_tensor("bout", [D], F32, kind="ExternalInput")
    out = nc.dram_tensor("out", [RC, D], F32, kind="ExternalOutput")

    KT_TILES = T // 128  # 16 key tiles

    with tile.TileContext(nc) as tc:
        with (
            tc.tile_pool(name="const", bufs=1) as const,
            tc.tile_pool(name="att", bufs=1) as attp,
            tc.tile_pool(name="sb", bufs=2) as sb,
        ):
            ident = const.tile([128, 128], BF16, tag="ident")
            masks.make_identity(nc, ident[:])
            sums_sb = const.tile([1, H * RC], BF16, tag="sums")
            bproj_bc = const.tile([128, D], F32, tag="bproj")
            nc.sync.dma_start(bproj_bc[:], bproj.ap().partition_broadcast(128))
            ln2w_bc = const.tile([128, D], F32, tag="ln2w")
            nc.sync.dma_start(ln2w_bc[:], ln2w.ap().partition_broadcast(128))
            ln2b_bc = const.tile([128, D], F32, tag="ln2b")
            nc.sync.dma_start(ln2b_bc[:], ln2b.ap().partition_broadcast(128))
            bout_bc = const.tile([128, D], F32, tag="bout")
            nc.sync.dma_start(bout_bc[:], bout.ap().partition_broadcast(128))
            bfc_all = const.tile([128, DF // 128], F32, tag="bfc")
            nc.sync.dma_start(bfc_all[:], bfc.ap().rearrange("(m p) -> p m", p=128))

            attT = [attp.tile([128, RC], BF16, tag=f"attT{i}", name=f"attT{i}") for i in range(8)]

            # ---------------- attention ----------------
            with (
                tc.tile_pool(name="kv", bufs=1) as kv,
                tc.tile_pool(name="exps", bufs=17) as exps,
                tc.tile_pool(name="aps", bufs=3, space="PSUM") as aps,
                tc.tile_pool(name="avps", bufs=2, space="PSUM") as avps,
                tc.tile_pool(name="asb", bufs=3) as asb,
            ):
                qT_sb = [kv.tile([128, RC], BF16, tag=f"qT{i}", name=f"qTs{i}") for i in range(8)]
                kT_sb = [kv.tile([128, T], BF16, tag=f"kT{i}", name=f"kTs{i}") for i in range(8)]
                v_sb = [kv.tile([128, VA], BF16, tag=f"v{i}", name=f"vs{i}") for i in range(KT_TILES)]
                m_sb = [kv.tile([128, RC], BF16, tag=f"m{i}", name=f"ms{i}") for i in range(KT_TILES)]
                for i in range(8):
                    nc.sync.dma_start(qT_sb[i][:], qT[i * 128:(i + 1) * 128, :])
                    nc.scalar.dma_start(kT_sb[i][:], kT[i * 128:(i + 1) * 128, :])
                for i in range(KT_TILES):
                    nc.sync.dma_start(v_sb[i][:], vaug[i * 128:(i + 1) * 128, :])
                    nc.scalar.dma_start(m_sb[i][:], maskT[i * 128:(i + 1) * 128, :])

                for hp in range(H // 2):  # head pairs (2hp, 2hp+1)
                    e_tiles = []
                    for kt in range(KT_TILES):
                        s_ps = aps.tile([128, 2 * RC], F32, tag="s")
                        nc.tensor.matmul(
                            s_ps[:, 0:RC],
                            kT_sb[hp][0:64, kt * 128:(kt + 1) * 128],
                            qT_sb[hp][0:64, :],
                            start=True, stop=True, tile_position=(0, 0),
                        )
                        nc.tensor.matmul(
                            s_ps[:, RC:2 * RC],
                            kT_sb[hp][64:128, kt * 128:(kt + 1) * 128],
                            qT_sb[hp][64:128, :],
                            start=True, stop=True, tile_position=(64, 0),
                        )
                        e_sb = exps.tile([128, 2 * RC], BF16, tag="e")
                        nc.scalar.activation(e_sb[:], s_ps[:], AF.Exp,
                                             bias=0.0, scale=0.125)
                        # causal mask (multiplicative)
                        nc.vector.tensor_mul(e_sb[:, 0:RC], e_sb[:, 0:RC], m_sb[kt][:])
                        nc.vector.tensor_mul(e_sb[:, RC:2 * RC], e_sb[:, RC:2 * RC], m_sb[kt][:])
                        e_tiles.append(e_sb)
                    for half in range(2):
                        h = 2 * hp + half
                        hl = half * 64
                        av_ps = avps.tile([DH + 1, RC], F32, tag="av")
                        for kt in range(KT_TILES):
                            nc.tensor.matmul(
                                av_ps[:], v_sb[kt][:, h * 65:(h + 1) * 65],
                                e_tiles[kt][:, half * RC:(half + 1) * RC],
                                start=(kt == 0), stop=(kt == KT_TILES - 1),
                            )
                        with nc.allow_low_precision(reason="softmax denom in bf16"):
                            nc.vector.tensor_copy(sums_sb[0:1, h * RC:(h + 1) * RC], av_ps[DH:DH + 1, :])
                        nc.vector.tensor_copy(attT[hp][hl:hl + 64, :], av_ps[0:DH, :])

                # one reciprocal over all 16 denominator rows, then rescale
                with nc.allow_low_precision(reason="softmax denom recip in bf16"):
                    nc.vector.reciprocal(sums_sb[:], sums_sb[:])
                for h in range(H):
                    hp, hl = h // 2, (h % 2) * 64
                    rb_sb = asb.tile([128, RC], BF16, tag="rb")
                    nc.gpsimd.partition_broadcast(
                        rb_sb[:], sums_sb[0:1, h * RC:(h + 1) * RC])
                    nc.vector.tensor_mul(
                        attT[hp][hl:hl + 64, :], attT[hp][hl:hl + 64, :],
                        rb_sb[hl:hl + 64, :],
                    )

            # ---------------- proj + residual + LN2 ----------------
            x2p_cm = tc.tile_pool(name="x2p", bufs=1)
            x2p = x2p_cm.__enter__()
            x_sb = [x2p.tile([128, D], F32, tag=f"x{t}", name=f"x{t}") for t in range(4)]
            for t in range(4):
                nc.sync.dma_start(x_sb[t][:], x[t * 128:(t + 1) * 128, :])
            ln2T = [x2p.tile([128, RC], BF16, tag=f"ln2T{j}", name=f"ln2T{j}") for j in range(8)]
            x2_sb = [x2p.tile([128, D], F32, tag=f"x2{t}", name=f"x2{t}") for t in range(4)]
            with (
                tc.tile_pool(name="wpj", bufs=1) as wpj,
                tc.tile_pool(name="pps", bufs=3, space="PSUM") as pps,
                tc.tile_pool(name="tps", bufs=3, space="PSUM") as tps,
            ):
                wp_sb = [wpj.tile([128, D], BF16, tag=f"wp{i}", name=f"wp{i}") for i in range(8)]
                for i in range(8):
                    nc.sync.dma_start(wp_sb[i][:], wproj[i * 128:(i + 1) * 128, :])
                for qm in range(4):
                    for oc in range(2):
                        y_ps = pps.tile([128, 512], F32, tag="y")
                        for cc in range(8):
                            nc.tensor.matmul(
                                y_ps[:],
                                attT[cc][:, qm * 128:(qm + 1) * 128],
                                wp_sb[cc][:, oc * 512:(oc + 1) * 512],
                                start=(cc == 0), stop=(cc == 7),
                            )
                        sl = slice(oc * 512, (oc + 1) * 512)
                        nc.vector.tensor_add(x2_sb[qm][:, sl], y_ps[:], x_sb[qm][:, sl])
                        nc.vector.tensor_add(
                            x2_sb[qm][:, sl], x2_sb[qm][:, sl], bproj_bc[:, sl],
                        )
                    ln2n = _layer_norm_tiles(nc, sb, x2_sb[qm], ln2w_bc, ln2b_bc, BF16)
                    _pe_transpose(nc, tps, ln2T, ln2n, qm, ident)

            # ---------------- FFN ----------------
            with tc.tile_pool(name="g", bufs=1) as gp:
                g_sb = [gp.tile([128, RC], BF16, tag=f"g{i}", name=f"g{i}") for i in range(32)]
                with (
                    tc.tile_pool(name="wf", bufs=1) as wf,
                    tc.tile_pool(name="fps", bufs=4, space="PSUM") as fps,
                ):
                    wf_sb = [wf.tile([128, DF], BF16, tag=f"wf{i}", name=f"wfs{i}") for i in range(8)]
                    for i in range(8):
                        nc.sync.dma_start(wf_sb[i][:], wfc[i * 128:(i + 1) * 128, :])
                    for hm in range(32):
                        h_ps = fps.tile([128, RC], F32, tag="h")
                        for k in range(8):
                            nc.tensor.matmul(
                                h_ps[:], wf_sb[k][:, hm * 128:(hm + 1) * 128],
                                ln2T[k][:], start=(k == 0), stop=(k == 7),
                            )
                        nc.scalar.activation(
                            g_sb[hm][:], h_ps[:], AF.Gelu,
                            bias=bfc_all[:, hm:hm + 1], scale=1.0,
                        )

                with (
                    tc.tile_pool(name="wo", bufs=1) as wo,
                    tc.tile_pool(name="ops", bufs=4, space="PSUM") as ops,
                    tc.tile_pool(name="osb", bufs=3) as osb,
                ):
                    wo_sb = [wo.tile([128, D], BF16, tag=f"wo{i}", name=f"wos{i}") for i in range(32)]
                    for i in range(32):
                        nc.sync.dma_start(wo_sb[i][:], wout[i * 128:(i + 1) * 128, :])
                    for qm in range(4):
                        o_tile = osb.tile([128, D], F32, tag="o")
                        for oc in range(2):
                            o_ps = ops.tile([128, 512], F32, tag="ops")
                            for hh in range(32):
                                nc.tensor.matmul(
                                    o_ps[:],
                                    g_sb[hh][:, qm * 128:(qm + 1) * 128],
                                    wo_sb[hh][:, oc * 512:(oc + 1) * 512],
                                    start=(hh == 0), stop=(hh == 31),
                                )
                            sl = slice(oc * 512, (oc + 1) * 512)
                            nc.vector.tensor_add(o_tile[:, sl], o_ps[:], x2_sb[qm][:, sl])
                            nc.vector.tensor_add(o_tile[:, sl], o_tile[:, sl], bout_bc[:, sl])
                        nc.sync.dma_start(out[qm * 128:(qm + 1) * 128, :], o_tile[:])
            x2p_cm.__exit__(None, None, None)

    nc.compile()
    return nc


_CACHE = {}


def _get(name, builder):
    if name not in _CACHE:
        _CACHE[name] = builder()
    return _CACHE[name]


def kernel(x, ln1_w, ln1_b, ln2_w, ln2_b, w_attn, b_attn, w_proj, b_proj,
           w_fc, b_fc, w_out, b_out):
    x = np.asarray(x, np.float32)
    xf = np.ascontiguousarray(x.reshape(R, D))
    wattn_bf = np.asarray(w_attn, np.float32).astype(BF)
    cores = list(range(NCORE))

    nc_a = _get("a", _build_a)
    in_maps_a = [
        dict(
            x=xf[c * RC:(c + 1) * RC],
            ln1w=np.asarray(ln1_w, np.float32), ln1b=np.asarray(ln1_b, np.float32),
            wattn=wattn_bf, battn=np.asarray(b_attn, np.float32),
        )
        for c in cores
    ]
    res_a = run_bass_kernel_spmd(nc_a, in_maps_a, cores).results
    qkvT = [np.asarray(res_a[c]["qkvT"]) for c in cores]  # [3072, 512] bf16

    # host reassembly (pure data movement / layout)
    kT_b = [np.concatenate([qkvT[4 * b + r][D:2 * D] for r in range(4)], axis=1)
            for b in range(B)]                            # [1024, 2048] bf16
    vT_b = [np.concatenate([qkvT[4 * b + r][2 * D:3 * D] for r in range(4)], axis=1)
            for b in range(B)]
    vaug_b = []
    for b in range(B):
        vn = np.ascontiguousarray(vT_b[b].T)              # [2048, 1024]
        va = np.empty((T, H, DH + 1), dtype=BF)
        va[:, :, :DH] = vn.reshape(T, H, DH)
        va[:, :, DH] = np.ones((), dtype=BF)
        vaug_b.append(va.reshape(T, VA))

    key_idx = np.arange(T)
    nc_b = _get("b", _build_b)
    in_maps_b = []
    for c in cores:
        b = c // 4
        qb = (c % 4) * 512 + np.arange(RC)                # batch-local query rows
        maskT = (key_idx[:, None] <= qb[None, :]).astype(BF)
        in_maps_b.append(dict(
            x=xf[c * RC:(c + 1) * RC],
            qT=np.ascontiguousarray(qkvT[c][0:D]),
            kT=np.ascontiguousarray(kT_b[b]),
            vaug=vaug_b[b],
            maskT=maskT,
            wproj=np.asarray(w_proj, np.float32).astype(BF),
            bproj=np.asarray(b_proj, np.float32),
            ln2w=np.asarray(ln2_w, np.float32), ln2b=np.asarray(ln2_b, np.float32),
            wfc=np.asarray(w_fc, np.float32).astype(BF),
            bfc=np.asarray(b_fc, np.float32),
            wout=np.asarray(w_out, np.float32).astype(BF),
            bout=np.asarray(b_out, np.float32),
        ))
    res_b = run_bass_kernel_spmd(nc_b, in_maps_b, cores).results
    out = np.concatenate([np.asarray(res_b[c]["out"], np.float32) for c in cores], axis=0)
    return out.reshape(B, T, D)
